# revision 29
# baseline (speedup 1.0000x reference)
"""Grouped GEMM (MoE routing) on 8 TRN2 NeuronCores.

Problem: out[off_g:off_g+size_g] = a[off_g:off_g+size_g] @ b[g] for 64 groups,
T=131072, K=1024, N=512, fp32. Group rows are contiguous in `a`.

Strategy (expert-parallel, host-specialized):
- Host reads the actual batch_sizes/offsets (numpy), LPT-balances the 64
  experts across 8 cores (132 tiles each for the reference sizes), then
  computes a static "segment" plan shared by all cores: the program is a
  flat list of NT 128-row tiles; segment j (static length L[j]) uses B
  buffer slot j, loaded from per-core input data. An expert may span
  multiple segments (its B is simply duplicated in the input), which lets
  NT approach the per-core ideal instead of the sum of per-rank maxima.
- A rows are packed + zero-padded into segment tile ranges, pre-transposed
  on host so matmul lhsT tiles load directly; DRAM layouts exactly match
  the SBUF tile layouts so DMA descriptors are 8KB-contiguous per
  partition (fast DGE + near-peak DMA).
- Matmul in fp16 (PSUM accumulates fp32, K=1024 over 8 chunks of 128).
  Output written back as fp16 (error << the fp32 roundtrip budget) to
  halve write traffic; host converts to fp32.
- A few dummy matmuls on a zeroed scratch tile warm the PE clock (HAM)
  during the initial DMA fill so real matmuls start at full rate.
"""

import sys

import numpy as np

sys.path.insert(0, "/opt/trn_rl_repo")

import concourse.tile as tile  # noqa: E402
from concourse import bacc, mybir  # noqa: E402
from concourse.bass_utils import run_bass_kernel_spmd  # noqa: E402

P = 128          # partitions / tile rows
K = 1024         # contraction dim
KC = K // P      # K chunks
NB = 512         # output columns
NCORES = 8
SBT = 8          # A tiles per superblock DMA (first block is 4)
SBT0 = 4         # first (gating) superblock size
IN_DT = mybir.dt.float16   # matmul input dtype (PSUM stays fp32)
OUT_DT = mybir.dt.float16  # DRAM output dtype (host converts to fp32)
NP_IN = np.float16
A_BUFS = 5
B_BUFS = 6
O_BUFS = 6
PS_BUFS = 7  # +1 bank reserved for the warmup dummy psum tile
NWARM = 6        # dummy matmuls to warm the PE during initial DMA fill
OBT = 4          # output tiles batched per DMA

_compiled = {}
last_results = None  # test harness introspection


# ---------------------------------------------------------------- planning

def _lpt_partition(n_g, ncores):
    """Balance experts across cores by tile count (largest first)."""
    import heapq
    h = [(0, c, ()) for c in range(ncores)]
    heapq.heapify(h)
    for g in sorted(range(len(n_g)), key=lambda g: -n_g[g]):
        s, c, lst = heapq.heappop(h)
        heapq.heappush(h, (s + int(n_g[g]), c, lst + (g,)))
    out = [None] * ncores
    for s, c, lst in h:
        out[c] = list(lst)
    return out


def _assign(L, exps, node_budget):
    """Assign expert sizes `exps` to disjoint subsets of segments with
    subset-sum >= size. Returns list of segment-index tuples (aligned with
    exps order) or None. DFS, minimal-waste-first."""
    order = sorted(range(len(exps)), key=lambda i: -exps[i])
    res = [None] * len(exps)
    cnt = [0]

    def dfs(oi, avail):
        cnt[0] += 1
        if cnt[0] > node_budget:
            return False
        if oi == len(order):
            return True
        need = exps[order[oi]]
        av = sorted(avail, key=lambda j: -L[j])
        if sum(L[j] for j in av) < sum(exps[order[i]] for i in range(oi, len(order))):
            return False
        cands = []
        for j in av:
            if L[j] >= need:
                cands.append((L[j] - need, (j,)))
        for x in range(len(av)):
            for y in range(x + 1, len(av)):
                s = L[av[x]] + L[av[y]]
                if s >= need:
                    cands.append((s - need, (av[x], av[y])))
        for x in range(len(av)):
            for y in range(x + 1, len(av)):
                for z in range(y + 1, len(av)):
                    s = L[av[x]] + L[av[y]] + L[av[z]]
                    if s >= need:
                        cands.append((s - need, (av[x], av[y], av[z])))
        cands.sort(key=lambda c: (c[0], len(c[1])))
        for _, sub in cands[:10]:
            res[order[oi]] = sub
            if dfs(oi + 1, avail - set(sub)):
                return True
        res[order[oi]] = None
        return False

    return res if dfs(0, frozenset(range(len(L)))) else None


def _plan(sizes):
    """Returns (cores, L, assigns): cores[c] = expert ids, L = static
    segment tile-lengths, assigns[c][i] = segment tuple for cores[c][i]."""
    import random
    n_g = [int(x) for x in (np.asarray(sizes) + P - 1) // P]
    cores = _lpt_partition(n_g, NCORES)
    multisets = [[n_g[g] for g in lst] for lst in cores]

    # Fallback: elementwise max over rank-sorted multisets (always feasible).
    smax = max(len(m) for m in multisets)
    base = [max((sorted(m, reverse=True) + [0] * smax)[i] for m in multisets)
            for i in range(smax)]
    base = [x for x in base if x > 0]

    def feasible(L):
        outs = []
        for ms in multisets:
            r = _assign(L, ms, 4000)
            if r is None:
                return None
            outs.append(r)
        return outs

    best_L, best_A = list(base), feasible(base)
    assert best_A is not None
    rnd = random.Random(12345)
    max_S = smax + 4
    for _ in range(3000):
        cand = list(best_L)
        op = rnd.random()
        if op < 0.4 and len(cand) > 1:
            j = rnd.randrange(len(cand))
            cand[j] -= rnd.randint(1, 3)
            if cand[j] <= 0:
                cand.pop(j)
        elif op < 0.7 and len(cand) < max_S:
            j = rnd.randrange(len(cand))
            if cand[j] >= 2:
                a = rnd.randint(1, cand[j] - 1)
                b = cand[j] - a - rnd.randint(0, 1)
                if b >= 1:
                    cand[j] = a
                    cand.append(b)
        else:
            j = rnd.randrange(len(cand))
            k = rnd.randrange(len(cand))
            if j != k and cand[j] > 1:
                cand[j] -= 1
        cand = [x for x in cand if x > 0]
        if not cand or len(cand) > max_S:
            continue
        key = (sum(cand), len(cand))
        if key >= (sum(best_L), len(best_L)):
            continue
        got = feasible(cand)
        if got is not None:
            best_L = sorted(cand, reverse=True)
            best_A = feasible(best_L)
    return cores, best_L, best_A


# ---------------------------------------------------------------- program

def _build_program(L):
    S = len(L)
    NT = sum(L)
    # A-block grid: first block SBT0 tiles (small gating transfer), rest SBT.
    nsb = 1 + max(0, -(-(NT - SBT0) // SBT))
    NT4 = SBT0 + (nsb - 1) * SBT
    blocks = [(0, min(SBT0, NT))] + [
        (SBT0 + i * SBT, min(SBT, NT - SBT0 - i * SBT)) for i in range(nsb - 1)]
    blk_start = {bs: (bi, cnt) for bi, (bs, cnt) in enumerate(blocks)}

    slot_of = []
    for s, ln in enumerate(L):
        slot_of += [s] * ln

    nob = (NT + OBT - 1) // OBT
    nc = bacc.Bacc("TRN2", target_bir_lowering=False, debug=False,
                   num_devices=NCORES)
    # DRAM layouts exactly match SBUF tile layouts: per-partition lines are
    # contiguous (8KB for A superblocks / B segments) -> efficient DMA.
    a_t0 = nc.dram_tensor("a_t0", [P, KC, SBT0 * P], IN_DT,
                          kind="ExternalInput").ap()
    a_t = nc.dram_tensor("a_t", [max(nsb - 1, 1), P, KC, SBT * P], IN_DT,
                         kind="ExternalInput").ap()
    b_p = nc.dram_tensor("b_p", [S, P, KC, NB], IN_DT,
                         kind="ExternalInput").ap()
    out = nc.dram_tensor("out", [nob, OBT, P, NB], OUT_DT,
                         kind="ExternalOutput").ap()

    with tile.TileContext(nc) as tc:
        with (
            tc.tile_pool(name="wpool", bufs=1) as wpool,
            tc.tile_pool(name="bpool", bufs=B_BUFS) as bpool,
            tc.tile_pool(name="apool", bufs=A_BUFS) as apool,
            tc.tile_pool(name="opool", bufs=O_BUFS) as opool,
            tc.tile_pool(name="psum", bufs=PS_BUFS, space="PSUM") as psum_pool,
            tc.tile_pool(name="wpsum", bufs=1, space="PSUM") as wpsum_pool,
        ):
            # Warm the PE (HAM clock gate) with dummy matmuls on a zeroed
            # scratch tile while the first A/B DMAs are in flight.
            w_sb = wpool.tile([P, NB], IN_DT)
            nc.vector.memset(w_sb[:], 0)
            ps_w = wpsum_pool.tile([P, NB], mybir.dt.float32)
            for _ in range(NWARM):
                nc.tensor.matmul(ps_w[:], w_sb[:, 0:P], w_sb[:],
                                 start=True, stop=True)

            b_slots = {}

            def load_b(s):
                b_sb = bpool.tile([P, KC, NB], IN_DT)
                nc.scalar.dma_start(b_sb[:], b_p[s])
                b_slots[s] = b_sb

            # Gating transfers (B0 + A-block0) as interleaved per-kc chunks
            # across both HWDGE rings: tile 0's kc-th matmul only needs the
            # kc-th chunks, so the PE can start as soon as the first chunks
            # land. Sync ring starts earlier -> it gets kc 0..4; scalar 5..7.
            h = KC // 2 + 1
            b0_sb = bpool.tile([P, KC, NB], IN_DT)
            a_sb = apool.tile([P, KC, SBT0 * P], IN_DT)
            for kc in range(h):
                nc.sync.dma_start(b0_sb[:, kc, :], b_p[0, :, kc, :])
                nc.sync.dma_start(a_sb[:, kc, :], a_t0[:, kc, :])
            for kc in range(h, KC):
                nc.scalar.dma_start(b0_sb[:, kc, :], b_p[0, :, kc, :])
                nc.scalar.dma_start(a_sb[:, kc, :], a_t0[:, kc, :])
            b_slots[0] = b0_sb
            for s in range(1, min(3, S)):
                load_b(s)
            o_sb = None
            cur_slot = 0
            bs = 0
            for t in range(NT):
                s = slot_of[t]
                if s != cur_slot:
                    cur_slot = s
                    if s + 2 < S:
                        load_b(s + 2)
                b_sb = b_slots[s]
                if t in blk_start and t > 0:
                    bi, cnt = blk_start[t]
                    bs = t
                    a_sb = apool.tile([P, KC, SBT * P], IN_DT)
                    nc.sync.dma_start(a_sb[:], a_t[bi - 1])
                ps = psum_pool.tile([P, NB], mybir.dt.float32)
                moff = (t - bs) * P
                for kc in range(KC):
                    nc.tensor.matmul(ps[:], a_sb[:, kc, moff:moff + P],
                                     b_sb[:, kc, :],
                                     start=(kc == 0), stop=(kc == KC - 1))
                if t % OBT == 0:
                    o_sb = opool.tile([P, OBT, NB], OUT_DT)
                nc.vector.tensor_copy(o_sb[:, t % OBT, :], ps[:])
                if t % OBT == OBT - 1 or t == NT - 1:
                    t0 = (t // OBT) * OBT
                    cnt = t - t0 + 1
                    nc.scalar.dma_start(
                        out[t // OBT, :cnt].rearrange("ti p n -> p ti n"),
                        o_sb[:, :cnt, :])
    nc.compile()
    return nc, NT, NT4, nsb


# ---------------------------------------------------------------- driver

def kernel(a, b, batch_sizes, batch_offsets, batch_padded_offsets):
    global last_results
    a = np.asarray(a, dtype=np.float32)
    b = np.asarray(b, dtype=np.float32)
    sizes = np.asarray(batch_sizes).astype(np.int64)
    offs = np.asarray(batch_offsets).astype(np.int64)
    T = a.shape[0]

    cores, L, assigns = _plan(sizes)
    key = tuple(L)
    if key not in _compiled:
        _compiled[key] = _build_program(L)
    nc, NT, NT4, nsb = _compiled[key]
    S = len(L)
    seg_tile0 = np.concatenate([[0], np.cumsum(L)])

    a16 = a.astype(NP_IN)
    b16 = b.astype(NP_IN)
    in_maps = []
    metas = []
    for c in range(NCORES):
        A_pad = np.zeros((NT4 * P, K), dtype=NP_IN)
        b_pc = np.zeros((S, P, KC, NB), dtype=NP_IN)
        meta = []
        for g, segs in zip(cores[c], assigns[c]):
            sz = int(sizes[g])
            off = int(offs[g])
            bg = np.ascontiguousarray(
                b16[g].reshape(KC, P, NB).transpose(1, 0, 2))
            pieces = []
            done = 0
            for j in sorted(segs):
                b_pc[j] = bg
                r0 = int(seg_tile0[j]) * P
                take = min(sz - done, L[j] * P)
                if take > 0:
                    A_pad[r0:r0 + take] = a16[off + done:off + done + take]
                    pieces.append((r0, take))
                    done += take
            assert done == sz, (done, sz)
            meta.append((off, sz, pieces))
        a_t0c = np.ascontiguousarray(
            A_pad[:SBT0 * P].reshape(SBT0 * P, KC, P).transpose(2, 1, 0))
        a_tc = np.ascontiguousarray(
            A_pad[SBT0 * P:].reshape(max(nsb - 1, 1), SBT * P, KC, P)
            .transpose(0, 3, 2, 1))
        in_maps.append({"a_t0": a_t0c, "a_t": a_tc, "b_p": b_pc})
        metas.append(meta)

    res = run_bass_kernel_spmd(nc, in_maps, list(range(NCORES)))
    last_results = res

    out = np.empty((T, NB), dtype=np.float32)
    for c in range(NCORES):
        oc = res.results[c]["out"].reshape(-1, NB)
        for (off, sz, pieces) in metas[c]:
            done = 0
            for (r0, take) in pieces:
                out[off + done:off + done + take] = oc[r0:r0 + take]
                done += take
    return out


# revision 30
# speedup vs baseline: 1.0404x; 1.0404x over previous
"""Grouped GEMM (MoE routing) on 8 TRN2 NeuronCores.

Problem: out[off_g:off_g+size_g] = a[off_g:off_g+size_g] @ b[g] for 64 groups,
T=131072, K=1024, N=512, fp32. Group rows are contiguous in `a`.

Strategy (expert-parallel, host-specialized):
- Host reads the actual batch_sizes/offsets (numpy), LPT-balances the 64
  experts across 8 cores (132 tiles each for the reference sizes), then
  computes a static "segment" plan shared by all cores: the program is a
  flat list of NT 128-row tiles; segment j (static length L[j]) uses B
  buffer slot j, loaded from per-core input data. An expert may span
  multiple segments (its B is simply duplicated in the input), which lets
  NT approach the per-core ideal instead of the sum of per-rank maxima.
- A rows are packed + zero-padded into segment tile ranges, pre-transposed
  on host so matmul lhsT tiles load directly; DRAM layouts exactly match
  the SBUF tile layouts so DMA descriptors are 8KB-contiguous per
  partition (fast DGE + near-peak DMA).
- Matmul in fp16 (PSUM accumulates fp32, K=1024 over 8 chunks of 128).
  Output written back as fp16 (error << the fp32 roundtrip budget) to
  halve write traffic; host converts to fp32.
- A few dummy matmuls on a zeroed scratch tile warm the PE clock (HAM)
  during the initial DMA fill so real matmuls start at full rate.
"""

import sys

import numpy as np

sys.path.insert(0, "/opt/trn_rl_repo")

import concourse.tile as tile  # noqa: E402
from concourse import bacc, mybir  # noqa: E402
from concourse.bass_utils import run_bass_kernel_spmd  # noqa: E402

P = 128          # partitions / tile rows
K = 1024         # contraction dim
KC = K // P      # K chunks
NB = 512         # output columns
NCORES = 8
SBT = 8          # A tiles per superblock DMA (first block is 4)
SBT0 = 4         # first (gating) superblock size
IN_DT = mybir.dt.float16   # matmul input dtype (PSUM stays fp32)
OUT_DT = mybir.dt.float16  # DRAM output dtype (host converts to fp32)
NP_IN = np.float16
A_BUFS = 5
B_BUFS = 6
O_BUFS = 6
PS_BUFS = 7  # +1 bank reserved for the warmup dummy psum tile
NWARM = 20       # dummy matmuls to warm the PE during initial DMA fill
OBT = 4          # output tiles batched per DMA

_compiled = {}
last_results = None  # test harness introspection


# ---------------------------------------------------------------- planning

def _lpt_partition(n_g, ncores):
    """Balance experts across cores by tile count (largest first)."""
    import heapq
    h = [(0, c, ()) for c in range(ncores)]
    heapq.heapify(h)
    for g in sorted(range(len(n_g)), key=lambda g: -n_g[g]):
        s, c, lst = heapq.heappop(h)
        heapq.heappush(h, (s + int(n_g[g]), c, lst + (g,)))
    out = [None] * ncores
    for s, c, lst in h:
        out[c] = list(lst)
    return out


def _assign(L, exps, node_budget):
    """Assign expert sizes `exps` to disjoint subsets of segments with
    subset-sum >= size. Returns list of segment-index tuples (aligned with
    exps order) or None. DFS, minimal-waste-first."""
    order = sorted(range(len(exps)), key=lambda i: -exps[i])
    res = [None] * len(exps)
    cnt = [0]

    def dfs(oi, avail):
        cnt[0] += 1
        if cnt[0] > node_budget:
            return False
        if oi == len(order):
            return True
        need = exps[order[oi]]
        av = sorted(avail, key=lambda j: -L[j])
        if sum(L[j] for j in av) < sum(exps[order[i]] for i in range(oi, len(order))):
            return False
        cands = []
        for j in av:
            if L[j] >= need:
                cands.append((L[j] - need, (j,)))
        for x in range(len(av)):
            for y in range(x + 1, len(av)):
                s = L[av[x]] + L[av[y]]
                if s >= need:
                    cands.append((s - need, (av[x], av[y])))
        for x in range(len(av)):
            for y in range(x + 1, len(av)):
                for z in range(y + 1, len(av)):
                    s = L[av[x]] + L[av[y]] + L[av[z]]
                    if s >= need:
                        cands.append((s - need, (av[x], av[y], av[z])))
        cands.sort(key=lambda c: (c[0], len(c[1])))
        for _, sub in cands[:10]:
            res[order[oi]] = sub
            if dfs(oi + 1, avail - set(sub)):
                return True
        res[order[oi]] = None
        return False

    return res if dfs(0, frozenset(range(len(L)))) else None


def _plan(sizes):
    """Returns (cores, L, assigns): cores[c] = expert ids, L = static
    segment tile-lengths, assigns[c][i] = segment tuple for cores[c][i]."""
    import random
    n_g = [int(x) for x in (np.asarray(sizes) + P - 1) // P]
    cores = _lpt_partition(n_g, NCORES)
    multisets = [[n_g[g] for g in lst] for lst in cores]

    # Fallback: elementwise max over rank-sorted multisets (always feasible).
    smax = max(len(m) for m in multisets)
    base = [max((sorted(m, reverse=True) + [0] * smax)[i] for m in multisets)
            for i in range(smax)]
    base = [x for x in base if x > 0]

    def feasible(L):
        outs = []
        for ms in multisets:
            r = _assign(L, ms, 4000)
            if r is None:
                return None
            outs.append(r)
        return outs

    best_L, best_A = list(base), feasible(base)
    assert best_A is not None
    rnd = random.Random(12345)
    max_S = smax + 4
    for _ in range(3000):
        cand = list(best_L)
        op = rnd.random()
        if op < 0.4 and len(cand) > 1:
            j = rnd.randrange(len(cand))
            cand[j] -= rnd.randint(1, 3)
            if cand[j] <= 0:
                cand.pop(j)
        elif op < 0.7 and len(cand) < max_S:
            j = rnd.randrange(len(cand))
            if cand[j] >= 2:
                a = rnd.randint(1, cand[j] - 1)
                b = cand[j] - a - rnd.randint(0, 1)
                if b >= 1:
                    cand[j] = a
                    cand.append(b)
        else:
            j = rnd.randrange(len(cand))
            k = rnd.randrange(len(cand))
            if j != k and cand[j] > 1:
                cand[j] -= 1
        cand = [x for x in cand if x > 0]
        if not cand or len(cand) > max_S:
            continue
        key = (sum(cand), len(cand))
        if key >= (sum(best_L), len(best_L)):
            continue
        got = feasible(cand)
        if got is not None:
            best_L = sorted(cand, reverse=True)
            best_A = feasible(best_L)
    return cores, best_L, best_A


# ---------------------------------------------------------------- program

def _build_program(L):
    S = len(L)
    NT = sum(L)
    # A-block grid: first block SBT0 tiles (small gating transfer), rest SBT.
    nsb = 1 + max(0, -(-(NT - SBT0) // SBT))
    NT4 = SBT0 + (nsb - 1) * SBT
    blocks = [(0, min(SBT0, NT))] + [
        (SBT0 + i * SBT, min(SBT, NT - SBT0 - i * SBT)) for i in range(nsb - 1)]
    blk_start = {bs: (bi, cnt) for bi, (bs, cnt) in enumerate(blocks)}

    slot_of = []
    for s, ln in enumerate(L):
        slot_of += [s] * ln

    nob = (NT + OBT - 1) // OBT
    nc = bacc.Bacc("TRN2", target_bir_lowering=False, debug=False,
                   num_devices=NCORES)
    # DRAM layouts exactly match SBUF tile layouts: per-partition lines are
    # contiguous (8KB for A superblocks / B segments) -> efficient DMA.
    a_t0 = nc.dram_tensor("a_t0", [P, KC, SBT0 * P], IN_DT,
                          kind="ExternalInput").ap()
    a_t = nc.dram_tensor("a_t", [max(nsb - 1, 1), P, KC, SBT * P], IN_DT,
                         kind="ExternalInput").ap()
    b_p = nc.dram_tensor("b_p", [S, P, KC, NB], IN_DT,
                         kind="ExternalInput").ap()
    out = nc.dram_tensor("out", [nob, OBT, P, NB], OUT_DT,
                         kind="ExternalOutput").ap()

    with tile.TileContext(nc) as tc:
        with (
            tc.tile_pool(name="wpool", bufs=1) as wpool,
            tc.tile_pool(name="bpool", bufs=B_BUFS) as bpool,
            tc.tile_pool(name="apool", bufs=A_BUFS) as apool,
            tc.tile_pool(name="opool", bufs=O_BUFS) as opool,
            tc.tile_pool(name="psum", bufs=PS_BUFS, space="PSUM") as psum_pool,
            tc.tile_pool(name="wpsum", bufs=1, space="PSUM") as wpsum_pool,
        ):
            # Warm the PE (HAM clock gate) with dummy matmuls on a zeroed
            # scratch tile while the first A/B DMAs are in flight.
            w_sb = wpool.tile([P, NB], IN_DT)
            nc.vector.memset(w_sb[:], 0)
            ps_w = wpsum_pool.tile([P, NB], mybir.dt.float32)
            for _ in range(NWARM):
                nc.tensor.matmul(ps_w[:], w_sb[:, 0:P], w_sb[:],
                                 start=True, stop=True)

            b_slots = {}

            def load_b(s):
                b_sb = bpool.tile([P, KC, NB], IN_DT)
                nc.scalar.dma_start(b_sb[:], b_p[s])
                b_slots[s] = b_sb

            # Gating transfers (B0 + A-block0) split across both HWDGE rings
            # so tile 0 is ready as early as possible; B1/B2 queue behind.
            h = KC // 2 + 1  # sync ring starts earlier; give it more
            b0_sb = bpool.tile([P, KC, NB], IN_DT)
            a_sb = apool.tile([P, KC, SBT0 * P], IN_DT)
            nc.sync.dma_start(b0_sb[:, :h, :], b_p[0, :, :h, :])
            nc.scalar.dma_start(b0_sb[:, h:, :], b_p[0, :, h:, :])
            nc.sync.dma_start(a_sb[:, :h, :], a_t0[:, :h, :])
            nc.scalar.dma_start(a_sb[:, h:, :], a_t0[:, h:, :])
            b_slots[0] = b0_sb
            for s in range(1, min(3, S)):
                load_b(s)
            o_sb = None
            cur_slot = 0
            bs = 0
            for t in range(NT):
                s = slot_of[t]
                if s != cur_slot:
                    cur_slot = s
                    if s + 2 < S:
                        load_b(s + 2)
                b_sb = b_slots[s]
                if t in blk_start and t > 0:
                    bi, cnt = blk_start[t]
                    bs = t
                    a_sb = apool.tile([P, KC, SBT * P], IN_DT)
                    nc.sync.dma_start(a_sb[:], a_t[bi - 1])
                ps = psum_pool.tile([P, NB], mybir.dt.float32)
                moff = (t - bs) * P
                for kc in range(KC):
                    nc.tensor.matmul(ps[:], a_sb[:, kc, moff:moff + P],
                                     b_sb[:, kc, :],
                                     start=(kc == 0), stop=(kc == KC - 1))
                if t % OBT == 0:
                    o_sb = opool.tile([P, OBT, NB], OUT_DT)
                nc.vector.tensor_copy(o_sb[:, t % OBT, :], ps[:])
                if t % OBT == OBT - 1 or t == NT - 1:
                    t0 = (t // OBT) * OBT
                    cnt = t - t0 + 1
                    nc.scalar.dma_start(
                        out[t // OBT, :cnt].rearrange("ti p n -> p ti n"),
                        o_sb[:, :cnt, :])
    nc.compile()
    return nc, NT, NT4, nsb


# ---------------------------------------------------------------- driver

def kernel(a, b, batch_sizes, batch_offsets, batch_padded_offsets):
    global last_results
    a = np.asarray(a, dtype=np.float32)
    b = np.asarray(b, dtype=np.float32)
    sizes = np.asarray(batch_sizes).astype(np.int64)
    offs = np.asarray(batch_offsets).astype(np.int64)
    T = a.shape[0]

    cores, L, assigns = _plan(sizes)
    key = tuple(L)
    if key not in _compiled:
        _compiled[key] = _build_program(L)
    nc, NT, NT4, nsb = _compiled[key]
    S = len(L)
    seg_tile0 = np.concatenate([[0], np.cumsum(L)])

    a16 = a.astype(NP_IN)
    b16 = b.astype(NP_IN)
    in_maps = []
    metas = []
    for c in range(NCORES):
        A_pad = np.zeros((NT4 * P, K), dtype=NP_IN)
        b_pc = np.zeros((S, P, KC, NB), dtype=NP_IN)
        meta = []
        for g, segs in zip(cores[c], assigns[c]):
            sz = int(sizes[g])
            off = int(offs[g])
            bg = np.ascontiguousarray(
                b16[g].reshape(KC, P, NB).transpose(1, 0, 2))
            pieces = []
            done = 0
            for j in sorted(segs):
                b_pc[j] = bg
                r0 = int(seg_tile0[j]) * P
                take = min(sz - done, L[j] * P)
                if take > 0:
                    A_pad[r0:r0 + take] = a16[off + done:off + done + take]
                    pieces.append((r0, take))
                    done += take
            assert done == sz, (done, sz)
            meta.append((off, sz, pieces))
        a_t0c = np.ascontiguousarray(
            A_pad[:SBT0 * P].reshape(SBT0 * P, KC, P).transpose(2, 1, 0))
        a_tc = np.ascontiguousarray(
            A_pad[SBT0 * P:].reshape(max(nsb - 1, 1), SBT * P, KC, P)
            .transpose(0, 3, 2, 1))
        in_maps.append({"a_t0": a_t0c, "a_t": a_tc, "b_p": b_pc})
        metas.append(meta)

    res = run_bass_kernel_spmd(nc, in_maps, list(range(NCORES)))
    last_results = res

    out = np.empty((T, NB), dtype=np.float32)
    for c in range(NCORES):
        oc = res.results[c]["out"].reshape(-1, NB)
        for (off, sz, pieces) in metas[c]:
            done = 0
            for (r0, take) in pieces:
                out[off + done:off + done + take] = oc[r0:r0 + take]
                done += take
    return out
